# revision 3
# baseline (speedup 1.0000x reference)
"""MoE layer (top-2 of 8 experts, exact GELU) on 8 Trainium2 NeuronCores.

Strategy: expert parallelism. The router (0.006% of the FLOPs) runs on the
host; each core holds one expert's weights resident in SBUF and runs the
two big matmuls for the tokens routed to that expert:

    H^T = W1[e]^T @ X_e^T          (PE, bf16, accumulated over C in fp32)
    G   = GELU(H^T + b1)           (ACT, fused bias, bf16 out)
    Y   = G^T @ W2[e]              (PE, bf16, accumulated over D in fp32)

The host dispatches each expert's token batch pre-transposed ([C, cap]) in
bf16, then combines Y across the two selected experts per token with the
renormalized top-2 weights (plus the b2 term) in fp32.

Computing H transposed (d on partitions) makes the first matmul's output
directly usable as the second matmul's stationary operand -- no on-device
transposes anywhere.
"""

import numpy as np
import ml_dtypes

B, T, C, D, E = 2, 2048, 1024, 4096, 8
N = B * T
TOP_K = 2
NT_BLOCK = 384          # token block width (matmul1 free dim)
TOK_TILE = 128          # token tile (matmul2 stationary free dim / psum partitions)
CB = 512                # output-channel tile (matmul2 free dim)

_cache = {}


MAX_WAITS = 1  # this walrus build rejects >1 sync wait per instruction


def _install_tile_patch():
    """This container's walrus build rejects >MAX_WAITS sync waits on one
    instruction ("Too many sync wait commands"). Two fixes:
    1. The stock TileContext tail puts every outstanding proc-clock wait on
       a single Drain -- split across sync-engine NOPs, one wait each.
    2. Body instructions can come out of sem assignment with 3+ waits --
       peel the excess onto same-engine NOPs inserted just before."""
    import bass_rust
    import concourse.mybir as mybir
    from concourse import tile
    from concourse.vector_clock import ScopedClock

    if getattr(tile.TileContext, "_drain_patched", False):
        return

    def _patched(self, tick_clock, wait_clock):
        nc = self.nc
        ticks = list(tick_clock.global_clock)
        for p, t in enumerate(ticks):
            if t <= 0:
                continue
            vc = bass_rust.VectorClock()
            vc.require_at_least(p, t)
            nop = nc.sync.nop(nofuse=True, hint=f"tail_wait_p{p}")
            wait_clock.add_sem_waits(nop.ins, ScopedClock({None: vc}))
        nc.sync.drain()
        nc.all_engine_barrier()
        assert self.sems is not None
        popped = nc._tile_sem_poison_stack.pop()
        assert popped is self._sem_poison
        nc.clear_and_free_semaphores(list(self.sems.allocated().values()))
        nc.all_engine_barrier()

    tile.TileContext._drain_and_barrier = _patched

    orig_lower = tile.TileContext._lower_ordered_insts

    def _split_excess_waits(self, ordered):
        nc = self.nc
        for bb_name, insts in ordered.items():
            out = []
            for inst in insts:
                si = getattr(inst, "sync_info", None)
                if (
                    si is not None
                    and si.on_wait
                    and len(si.on_wait) > MAX_WAITS
                    and isinstance(inst, mybir.Instruction)
                    and inst.engine != mybir.EngineType.Unassigned
                ):
                    waits = list(si.on_wait)
                    excess, keep = waits[:-MAX_WAITS], waits[-MAX_WAITS:]
                    while excess:
                        chunk, excess = excess[:MAX_WAITS], excess[MAX_WAITS:]
                        nop = mybir.InstNoOp(
                            name=nc.get_next_instruction_name(),
                            sync_info=mybir.SyncInfo(on_wait=chunk, on_update=[]),
                            bass_nofuse=True,
                            engine=inst.engine,
                        )
                        nc.register_instruction(nop)
                        out.append(nop)
                    inst.sync_info = mybir.SyncInfo(
                        on_wait=keep, on_update=list(si.on_update or [])
                    )
                out.append(inst)
            insts[:] = out
        return orig_lower(self, ordered)

    tile.TileContext._lower_ordered_insts = _split_excess_waits
    tile.TileContext._drain_patched = True


def _build(cap):
    """Build the per-core Bass program for token capacity `cap`
    (a multiple of NT_BLOCK)."""
    import concourse.bass as bass
    import concourse.mybir as mybir
    import concourse.tile as tile
    from contextlib import ExitStack

    _install_tile_patch()

    bf16 = mybir.dt.bfloat16
    f32 = mybir.dt.float32
    KC = C // 128            # 8 contraction chunks for matmul1
    DT = D // 128            # 32 d-tiles / contraction chunks for matmul2
    nblocks = cap // NT_BLOCK

    nc = bass.Bass()
    xt = nc.declare_dram_parameter("xt", [KC, 128, cap], bf16, isOutput=False)
    w1 = nc.declare_dram_parameter("w1", [KC, 128, D], bf16, isOutput=False)
    w2 = nc.declare_dram_parameter("w2", [DT, 128, C], bf16, isOutput=False)
    b1t = nc.declare_dram_parameter("b1t", [128, DT], f32, isOutput=False)
    y = nc.declare_dram_parameter("y", [cap, C], f32, isOutput=True)

    with tile.TileContext(nc) as tc, ExitStack() as ctx:
        const = ctx.enter_context(tc.tile_pool(name="const", bufs=1))
        w1s = const.tile([128, KC, D], bf16)
        w2s = const.tile([128, DT, C], bf16)
        xts = const.tile([128, KC, cap], bf16)
        b1s = const.tile([128, DT], f32)

        # Order matters only as scheduling priority: matmul1 needs xt+w1
        # first; w2 streams in while matmul1 runs.
        for kc in range(KC):
            nc.sync.dma_start(xts[:, kc, :], xt[kc])
        nc.sync.dma_start(b1s[:], b1t[:])
        for kc in range(KC):
            nc.sync.dma_start(w1s[:, kc, :], w1[kc])
        for dt in range(DT):
            nc.sync.dma_start(w2s[:, dt, :], w2[dt])

        gpool = ctx.enter_context(tc.tile_pool(name="g", bufs=2))
        ps1 = ctx.enter_context(tc.tile_pool(name="ps1", bufs=4, space="PSUM"))
        ps2 = ctx.enter_context(tc.tile_pool(name="ps2", bufs=4, space="PSUM"))
        yev = ctx.enter_context(tc.tile_pool(name="yev", bufs=4))

        for blk in range(nblocks):
            t0 = blk * NT_BLOCK
            g = gpool.tile([128, DT, NT_BLOCK], bf16)
            for dt in range(DT):
                ph = ps1.tile([128, NT_BLOCK], f32)
                for kc in range(KC):
                    nc.tensor.matmul(
                        ph[:],
                        w1s[:, kc, 128 * dt : 128 * (dt + 1)],
                        xts[:, kc, t0 : t0 + NT_BLOCK],
                        start=(kc == 0),
                        stop=(kc == KC - 1),
                    )
                nc.scalar.activation(
                    g[:, dt, :], ph[:],
                    mybir.ActivationFunctionType.Gelu,
                    bias=b1s[:, dt : dt + 1],
                )
            for tt in range(NT_BLOCK // TOK_TILE):
                for cb in range(C // CB):
                    py = ps2.tile([128, CB], f32)
                    for dt in range(DT):
                        nc.tensor.matmul(
                            py[:],
                            g[:, dt, TOK_TILE * tt : TOK_TILE * (tt + 1)],
                            w2s[:, dt, CB * cb : CB * (cb + 1)],
                            start=(dt == 0),
                            stop=(dt == DT - 1),
                        )
                    yt = yev.tile([128, CB], f32)
                    nc.vector.tensor_copy(yt[:], py[:])
                    r0 = t0 + TOK_TILE * tt
                    nc.sync.dma_start(
                        y[r0 : r0 + TOK_TILE, CB * cb : CB * (cb + 1)], yt[:]
                    )
    return nc


def _route(xf, w_router):
    """Host router: softmax over experts, top-2 (jax tie semantics:
    stable, lower index first), renormalize."""
    logits = xf @ w_router.T                       # [N, E] fp32
    m = logits.max(axis=-1, keepdims=True)
    p = np.exp(logits - m)
    p /= p.sum(axis=-1, keepdims=True)
    topi = np.argsort(-p, axis=-1, kind="stable")[:, :TOP_K]   # [N, 2]
    topw = np.take_along_axis(p, topi, axis=-1)
    topw = topw / topw.sum(axis=-1, keepdims=True)
    return topi.astype(np.int32), topw.astype(np.float32)


def _run_spmd(nc, in_maps, trace=False, trace_cores=None, tmpdir=None):
    from concourse.bass_utils import run_bass_kernel_spmd

    return run_bass_kernel_spmd(
        nc, in_maps, core_ids=list(range(E)),
        trace=trace, trace_cores=trace_cores, tmpdir=tmpdir,
    )


# test.py hooks: set TRACE=True (and optionally TRACE_CORES/TRACE_DIR)
# before calling kernel() to capture an NTFF profile of the run.
TRACE = False
TRACE_CORES = None
TRACE_DIR = None
LAST_RESULT = None


def kernel(x, w_router, w1, b1, w2, b2):
    global LAST_RESULT
    x = np.asarray(x, dtype=np.float32)
    w_router = np.asarray(w_router, dtype=np.float32)
    w1 = np.asarray(w1, dtype=np.float32)
    b1 = np.asarray(b1, dtype=np.float32)
    w2 = np.asarray(w2, dtype=np.float32)
    b2 = np.asarray(b2, dtype=np.float32)

    xf = x.reshape(N, C)
    topi, topw = _route(xf, w_router)

    # token rows routed to each expert (each token appears in exactly 2)
    sel = [np.nonzero((topi == e).any(axis=-1))[0] for e in range(E)]
    max_cnt = max(len(s) for s in sel)
    cap = max(NT_BLOCK, -(-max_cnt // NT_BLOCK) * NT_BLOCK)

    if cap not in _cache:
        _cache[cap] = _build(cap)
    nc = _cache[cap]

    bf16 = ml_dtypes.bfloat16
    xf_bf = xf.astype(bf16)
    in_maps = []
    for e in range(E):
        rows = sel[e]
        xt = np.zeros((C, cap), dtype=bf16)
        xt[:, : len(rows)] = xf_bf[rows].T
        in_maps.append({
            "xt": np.ascontiguousarray(xt.reshape(C // 128, 128, cap)),
            "w1": np.ascontiguousarray(w1[e].astype(bf16).reshape(C // 128, 128, D)),
            "w2": np.ascontiguousarray(w2[e].astype(bf16).reshape(D // 128, 128, C)),
            "b1t": np.ascontiguousarray(b1[e].reshape(D // 128, 128).T),
        })

    res = _run_spmd(nc, in_maps, trace=TRACE, trace_cores=TRACE_CORES,
                    tmpdir=TRACE_DIR)
    LAST_RESULT = res

    out = np.zeros((N, C), dtype=np.float32)
    for e in range(E):
        rows = sel[e]
        if len(rows) == 0:
            continue
        ye = np.asarray(res.results[e]["y"], dtype=np.float32)[: len(rows)]
        # weight of expert e for each selected token
        is_e = topi[rows] == e               # [n_e, 2]
        wgt = (topw[rows] * is_e).sum(axis=-1)
        out[rows] += wgt[:, None] * ye
    # b2 enters after the expert matmul, inside the weighted combine
    out += (topw[:, :, None] * b2[topi]).sum(axis=1)
    return out.reshape(B, T, C)


# revision 7
# speedup vs baseline: 1.0160x; 1.0160x over previous
"""MoE layer (top-2 of 8 experts, exact GELU) on 8 Trainium2 NeuronCores.

Strategy: expert parallelism. The router (0.006% of the FLOPs) runs on the
host; each core holds one expert's weights resident in SBUF and runs the
two big matmuls for the tokens routed to that expert:

    H^T = W1[e]^T @ X_e^T          (PE, bf16, accumulated over C in fp32)
    G   = GELU(H^T + b1)           (ACT, fused bias, bf16 out)
    Y   = G^T @ W2[e]              (PE, bf16, accumulated over D in fp32)

The host dispatches each expert's token batch pre-transposed ([C, cap]) in
bf16, then combines Y across the two selected experts per token with the
renormalized top-2 weights (plus the b2 term) in fp32.

Computing H transposed (d on partitions) makes the first matmul's output
directly usable as the second matmul's stationary operand -- no on-device
transposes anywhere.
"""

import numpy as np
import ml_dtypes

B, T, C, D, E = 2, 2048, 1024, 4096, 8
N = B * T
TOP_K = 2
NT_BLOCK = 384          # token block width (matmul1 free dim)
TOK_TILE = 128          # token tile (matmul2 stationary free dim / psum partitions)
CB = 512                # output-channel tile (matmul2 free dim)

_cache = {}


MAX_WAITS = 1  # this walrus build rejects >1 sync wait per instruction


def _install_tile_patch():
    """This container's walrus build rejects >MAX_WAITS sync waits on one
    instruction ("Too many sync wait commands"). Two fixes:
    1. The stock TileContext tail puts every outstanding proc-clock wait on
       a single Drain -- split across sync-engine NOPs, one wait each.
    2. Body instructions can come out of sem assignment with 3+ waits --
       peel the excess onto same-engine NOPs inserted just before."""
    import bass_rust
    import concourse.mybir as mybir
    from concourse import tile
    from concourse.vector_clock import ScopedClock

    if getattr(tile.TileContext, "_drain_patched", False):
        return

    def _patched(self, tick_clock, wait_clock):
        nc = self.nc
        ticks = list(tick_clock.global_clock)
        for p, t in enumerate(ticks):
            if t <= 0:
                continue
            vc = bass_rust.VectorClock()
            vc.require_at_least(p, t)
            nop = nc.sync.nop(nofuse=True, hint=f"tail_wait_p{p}")
            wait_clock.add_sem_waits(nop.ins, ScopedClock({None: vc}))
        nc.sync.drain()
        nc.all_engine_barrier()
        assert self.sems is not None
        popped = nc._tile_sem_poison_stack.pop()
        assert popped is self._sem_poison
        nc.clear_and_free_semaphores(list(self.sems.allocated().values()))
        nc.all_engine_barrier()

    tile.TileContext._drain_and_barrier = _patched

    orig_lower = tile.TileContext._lower_ordered_insts

    def _split_excess_waits(self, ordered):
        nc = self.nc
        for bb_name, insts in ordered.items():
            out = []
            for inst in insts:
                si = getattr(inst, "sync_info", None)
                if (
                    si is not None
                    and si.on_wait
                    and len(si.on_wait) > MAX_WAITS
                    and isinstance(inst, mybir.Instruction)
                    and inst.engine != mybir.EngineType.Unassigned
                ):
                    waits = list(si.on_wait)
                    excess, keep = waits[:-MAX_WAITS], waits[-MAX_WAITS:]
                    while excess:
                        chunk, excess = excess[:MAX_WAITS], excess[MAX_WAITS:]
                        nop = mybir.InstNoOp(
                            name=nc.get_next_instruction_name(),
                            sync_info=mybir.SyncInfo(on_wait=chunk, on_update=[]),
                            bass_nofuse=True,
                            engine=inst.engine,
                        )
                        nc.register_instruction(nop)
                        out.append(nop)
                    inst.sync_info = mybir.SyncInfo(
                        on_wait=keep, on_update=list(si.on_update or [])
                    )
                out.append(inst)
            insts[:] = out
        return orig_lower(self, ordered)

    tile.TileContext._lower_ordered_insts = _split_excess_waits
    tile.TileContext._drain_patched = True


def _blocks_of(cap):
    """Token-block widths covering cap (multiple of 64). Full NT_BLOCK
    blocks plus one remainder block."""
    blocks = [NT_BLOCK] * (cap // NT_BLOCK)
    if cap % NT_BLOCK:
        blocks.append(cap % NT_BLOCK)
    return blocks


def _tok_tiles(width):
    """Split a block width into matmul2 stationary tiles (<=128 each)."""
    tiles = [TOK_TILE] * (width // TOK_TILE)
    if width % TOK_TILE:
        tiles.append(width % TOK_TILE)
    return tiles


def _build(cap):
    """Build the per-core Bass program for token capacity `cap`
    (a multiple of 64)."""
    import concourse.bass as bass
    import concourse.mybir as mybir
    import concourse.tile as tile
    from contextlib import ExitStack

    _install_tile_patch()

    bf16 = mybir.dt.bfloat16
    f32 = mybir.dt.float32
    KC = C // 128            # 8 contraction chunks for matmul1
    DT = D // 128            # 32 d-tiles / contraction chunks for matmul2
    WG = 512                 # w1 d-columns per DMA group (4 d-tiles)
    NG = D // WG             # 8 groups
    blocks = _blocks_of(cap)

    nc = bass.Bass()
    # w1 is staged dt-group-major so matmul1 can start after one group.
    xt = nc.declare_dram_parameter("xt", [KC, 128, cap], bf16, isOutput=False)
    w1 = nc.declare_dram_parameter("w1", [NG, KC, 128, WG], bf16, isOutput=False)
    w2 = nc.declare_dram_parameter("w2", [DT, 128, C], bf16, isOutput=False)
    b1t = nc.declare_dram_parameter("b1t", [128, DT], f32, isOutput=False)
    y = nc.declare_dram_parameter("y", [cap, C], f32, isOutput=True)

    with tile.TileContext(nc) as tc, ExitStack() as ctx:
        const = ctx.enter_context(tc.tile_pool(name="const", bufs=1))
        w1s = const.tile([128, KC, D], bf16)
        w2s = const.tile([128, DT, C], bf16)
        xts = const.tile([128, KC, cap], bf16)
        b1s = const.tile([128, DT], f32)

        # DMA emission order == consumption order (priority for the
        # scheduler): bias, block-0 activations, w1 by d-group (matmul1
        # consumes d-major), remaining activations, then w2 (consumed
        # dt-streaming by matmul2's dt-outer loop).
        nc.sync.dma_start(b1s[:], b1t[:])
        t0 = 0
        for kc in range(KC):
            nc.sync.dma_start(xts[:, kc, : blocks[0]], xt[kc, :, : blocks[0]])
        for g in range(NG):
            for kc in range(KC):
                nc.sync.dma_start(w1s[:, kc, WG * g : WG * (g + 1)], w1[g, kc])
        for blk in range(1, len(blocks)):
            t0 += blocks[blk - 1]
            for kc in range(KC):
                nc.sync.dma_start(
                    xts[:, kc, t0 : t0 + blocks[blk]],
                    xt[kc, :, t0 : t0 + blocks[blk]],
                )
        for dt in range(DT):
            nc.sync.dma_start(w2s[:, dt, :], w2[dt])

        gpool = ctx.enter_context(tc.tile_pool(name="g", bufs=2))
        ps1 = ctx.enter_context(tc.tile_pool(name="ps1", bufs=2, space="PSUM"))
        ps2 = ctx.enter_context(tc.tile_pool(name="ps2", bufs=6, space="PSUM"))
        yev = ctx.enter_context(tc.tile_pool(name="yev", bufs=4))

        t0 = 0
        for blk, bw in enumerate(blocks):
            g = gpool.tile([128, DT, bw], bf16, tag="g")
            for dt in range(DT):
                ph = ps1.tile([128, bw], f32, tag="ph")
                for kc in range(KC):
                    nc.tensor.matmul(
                        ph[:],
                        w1s[:, kc, 128 * dt : 128 * (dt + 1)],
                        xts[:, kc, t0 : t0 + bw],
                        start=(kc == 0),
                        stop=(kc == KC - 1),
                    )
                nc.scalar.activation(
                    g[:, dt, :], ph[:],
                    mybir.ActivationFunctionType.Gelu,
                    bias=b1s[:, dt : dt + 1],
                )
            # matmul2, dt outermost: 6 concurrent PSUM accumulators, so w2
            # chunks are consumed in DMA order and g[dt] right after its GELU.
            tts = _tok_tiles(bw)
            accs = []
            for tt, tw in enumerate(tts):
                for cb in range(C // CB):
                    accs.append((
                        ps2.tile([128, CB], f32, tag="py",
                                 name=f"py_b{blk}_t{tt}_c{cb}"),
                        tt, tw, cb,
                    ))
            for dt in range(DT):
                for py, tt, tw, cb in accs:
                    nc.tensor.matmul(
                        py[:tw, :],
                        g[:, dt, TOK_TILE * tt : TOK_TILE * tt + tw],
                        w2s[:, dt, CB * cb : CB * (cb + 1)],
                        start=(dt == 0),
                        stop=(dt == DT - 1),
                    )
            for py, tt, tw, cb in accs:
                yt = yev.tile([128, CB], f32, tag="yt")
                nc.vector.tensor_copy(yt[:tw, :], py[:tw, :])
                r0 = t0 + TOK_TILE * tt
                nc.sync.dma_start(
                    y[r0 : r0 + tw, CB * cb : CB * (cb + 1)], yt[:tw, :]
                )
            t0 += bw
    return nc


def _route(xf, w_router):
    """Host router: softmax over experts, top-2 (jax tie semantics:
    stable, lower index first), renormalize."""
    logits = xf @ w_router.T                       # [N, E] fp32
    m = logits.max(axis=-1, keepdims=True)
    p = np.exp(logits - m)
    p /= p.sum(axis=-1, keepdims=True)
    topi = np.argsort(-p, axis=-1, kind="stable")[:, :TOP_K]   # [N, 2]
    topw = np.take_along_axis(p, topi, axis=-1)
    topw = topw / topw.sum(axis=-1, keepdims=True)
    return topi.astype(np.int32), topw.astype(np.float32)


def _run_spmd(nc, in_maps, trace=False, trace_cores=None, tmpdir=None):
    from concourse.bass_utils import run_bass_kernel_spmd

    return run_bass_kernel_spmd(
        nc, in_maps, core_ids=list(range(E)),
        trace=trace, trace_cores=trace_cores, tmpdir=tmpdir,
    )


# test.py hooks: set TRACE=True (and optionally TRACE_CORES/TRACE_DIR)
# before calling kernel() to capture an NTFF profile of the run.
TRACE = False
TRACE_CORES = None
TRACE_DIR = None
LAST_RESULT = None


def kernel(x, w_router, w1, b1, w2, b2):
    global LAST_RESULT
    x = np.asarray(x, dtype=np.float32)
    w_router = np.asarray(w_router, dtype=np.float32)
    w1 = np.asarray(w1, dtype=np.float32)
    b1 = np.asarray(b1, dtype=np.float32)
    w2 = np.asarray(w2, dtype=np.float32)
    b2 = np.asarray(b2, dtype=np.float32)

    xf = x.reshape(N, C)
    topi, topw = _route(xf, w_router)

    # token rows routed to each expert (each token appears in exactly 2)
    sel = [np.nonzero((topi == e).any(axis=-1))[0] for e in range(E)]
    max_cnt = max(len(s) for s in sel)
    cap = max(128, -(-max_cnt // 64) * 64)

    if cap not in _cache:
        _cache[cap] = _build(cap)
    nc = _cache[cap]

    bf16 = ml_dtypes.bfloat16
    xf_bf = xf.astype(bf16)
    in_maps = []
    for e in range(E):
        rows = sel[e]
        xt = np.zeros((C, cap), dtype=bf16)
        xt[:, : len(rows)] = xf_bf[rows].T
        # w1 staged as [d-group, kc, 128, 512] (see _build)
        w1t = w1[e].astype(bf16).reshape(C // 128, 128, D // 512, 512)
        w1t = np.ascontiguousarray(w1t.transpose(2, 0, 1, 3))
        in_maps.append({
            "xt": np.ascontiguousarray(xt.reshape(C // 128, 128, cap)),
            "w1": w1t,
            "w2": np.ascontiguousarray(w2[e].astype(bf16).reshape(D // 128, 128, C)),
            "b1t": np.ascontiguousarray(b1[e].reshape(D // 128, 128).T),
        })

    res = _run_spmd(nc, in_maps, trace=TRACE, trace_cores=TRACE_CORES,
                    tmpdir=TRACE_DIR)
    LAST_RESULT = res

    out = np.zeros((N, C), dtype=np.float32)
    for e in range(E):
        rows = sel[e]
        if len(rows) == 0:
            continue
        ye = np.asarray(res.results[e]["y"], dtype=np.float32)[: len(rows)]
        # weight of expert e for each selected token
        is_e = topi[rows] == e               # [n_e, 2]
        wgt = (topw[rows] * is_e).sum(axis=-1)
        out[rows] += wgt[:, None] * ye
    # b2 enters after the expert matmul, inside the weighted combine
    out += (topw[:, :, None] * b2[topi]).sum(axis=1)
    return out.reshape(B, T, C)


# revision 10
# speedup vs baseline: 1.0623x; 1.0456x over previous
"""MoE layer (top-2 of 8 experts, exact GELU) on 8 Trainium2 NeuronCores.

Strategy: expert parallelism. The router (0.006% of the FLOPs) runs on the
host; each core holds one expert's weights resident in SBUF and runs the
two big matmuls for the tokens routed to that expert:

    H^T = W1[e]^T @ X_e^T          (PE, bf16, accumulated over C in fp32)
    G   = GELU(H^T + b1)           (ACT, fused bias, bf16 out)
    Y   = G^T @ W2[e]              (PE, bf16, accumulated over D in fp32)

The host dispatches each expert's token batch pre-transposed ([C, cap]) in
bf16, then combines Y across the two selected experts per token with the
renormalized top-2 weights (plus the b2 term) in fp32.

Computing H transposed (d on partitions) makes the first matmul's output
directly usable as the second matmul's stationary operand -- no on-device
transposes anywhere.
"""

import numpy as np
import ml_dtypes

B, T, C, D, E = 2, 2048, 1024, 4096, 8
N = B * T
TOP_K = 2
NT_BLOCK = 384          # token block width (matmul1 free dim)
TOK_TILE = 128          # token tile (matmul2 stationary free dim / psum partitions)
CB = 512                # output-channel tile (matmul2 free dim)

_cache = {}


MAX_WAITS = 1  # this walrus build rejects >1 sync wait per instruction


def _install_tile_patch():
    """This container's walrus build rejects >MAX_WAITS sync waits on one
    instruction ("Too many sync wait commands"). Two fixes:
    1. The stock TileContext tail puts every outstanding proc-clock wait on
       a single Drain -- split across sync-engine NOPs, one wait each.
    2. Body instructions can come out of sem assignment with 3+ waits --
       peel the excess onto same-engine NOPs inserted just before."""
    import bass_rust
    import concourse.mybir as mybir
    from concourse import tile
    from concourse.vector_clock import ScopedClock

    if getattr(tile.TileContext, "_drain_patched", False):
        return

    def _patched(self, tick_clock, wait_clock):
        nc = self.nc
        ticks = list(tick_clock.global_clock)
        for p, t in enumerate(ticks):
            if t <= 0:
                continue
            vc = bass_rust.VectorClock()
            vc.require_at_least(p, t)
            nop = nc.sync.nop(nofuse=True, hint=f"tail_wait_p{p}")
            wait_clock.add_sem_waits(nop.ins, ScopedClock({None: vc}))
        nc.sync.drain()
        nc.all_engine_barrier()
        assert self.sems is not None
        popped = nc._tile_sem_poison_stack.pop()
        assert popped is self._sem_poison
        nc.clear_and_free_semaphores(list(self.sems.allocated().values()))
        nc.all_engine_barrier()

    tile.TileContext._drain_and_barrier = _patched

    orig_lower = tile.TileContext._lower_ordered_insts

    def _split_excess_waits(self, ordered):
        nc = self.nc
        for bb_name, insts in ordered.items():
            out = []
            for inst in insts:
                si = getattr(inst, "sync_info", None)
                if (
                    si is not None
                    and si.on_wait
                    and len(si.on_wait) > MAX_WAITS
                    and isinstance(inst, mybir.Instruction)
                    and inst.engine != mybir.EngineType.Unassigned
                ):
                    waits = list(si.on_wait)
                    excess, keep = waits[:-MAX_WAITS], waits[-MAX_WAITS:]
                    while excess:
                        chunk, excess = excess[:MAX_WAITS], excess[MAX_WAITS:]
                        nop = mybir.InstNoOp(
                            name=nc.get_next_instruction_name(),
                            sync_info=mybir.SyncInfo(on_wait=chunk, on_update=[]),
                            bass_nofuse=True,
                            engine=inst.engine,
                        )
                        nc.register_instruction(nop)
                        out.append(nop)
                    inst.sync_info = mybir.SyncInfo(
                        on_wait=keep, on_update=list(si.on_update or [])
                    )
                out.append(inst)
            insts[:] = out
        return orig_lower(self, ordered)

    tile.TileContext._lower_ordered_insts = _split_excess_waits
    tile.TileContext._drain_patched = True


def _blocks_of(cap):
    """Token-block widths covering cap (multiple of 64). Full NT_BLOCK
    blocks plus one remainder block."""
    blocks = [NT_BLOCK] * (cap // NT_BLOCK)
    if cap % NT_BLOCK:
        blocks.append(cap % NT_BLOCK)
    return blocks


def _tok_tiles(width):
    """Split a block width into matmul2 stationary tiles (<=128 each)."""
    tiles = [TOK_TILE] * (width // TOK_TILE)
    if width % TOK_TILE:
        tiles.append(width % TOK_TILE)
    return tiles


def _build(cap):
    """Build the per-core Bass program for token capacity `cap`
    (a multiple of 64)."""
    import concourse.bass as bass
    import concourse.mybir as mybir
    import concourse.tile as tile
    from contextlib import ExitStack

    _install_tile_patch()

    bf16 = mybir.dt.bfloat16
    f32 = mybir.dt.float32
    KC = C // 128            # 8 contraction chunks for matmul1
    DT = D // 128            # 32 d-tiles / contraction chunks for matmul2
    WG = 512                 # w1 d-columns per DMA group (4 d-tiles)
    NG = D // WG             # 8 groups
    blocks = _blocks_of(cap)

    nc = bass.Bass()
    # Inputs are host-pre-tiled so every DMA is one big partition-major
    # transfer (~1 MB): small DMAs serialize at ~600 ns each on the sync
    # queue and starve the PE.
    #   xt : [128, KC, cap]   xt[p, kc, t]  = x^T[kc*128+p, t]
    #   w1 : [NG, 128, KC, WG] w1[g, p, kc, j] = w1[kc*128+p, g*WG+j]
    #   w2 : [NQ, 128, 4, C]  w2[q, p, a, c] = w2[(4q+a)*128+p, c]
    NQ = DT // 4
    xt = nc.declare_dram_parameter("xt", [128, KC, cap], bf16, isOutput=False)
    w1 = nc.declare_dram_parameter("w1", [NG, 128, KC, WG], bf16, isOutput=False)
    w2 = nc.declare_dram_parameter("w2", [NQ, 128, 4, C], bf16, isOutput=False)
    b1t = nc.declare_dram_parameter("b1t", [128, DT], f32, isOutput=False)
    y = nc.declare_dram_parameter("y", [cap, C], bf16, isOutput=True)

    with tile.TileContext(nc) as tc, ExitStack() as ctx:
        const = ctx.enter_context(tc.tile_pool(name="const", bufs=1))
        w1s = const.tile([128, KC, D], bf16)
        w2s = const.tile([128, DT, C], bf16)
        xts = const.tile([128, KC, cap], bf16)
        b1s = const.tile([128, DT], f32)

        # DMA emission order == consumption order (priority for the
        # scheduler): bias, block-0 activations, w1 by d-group (matmul1
        # consumes d-major), remaining activations, then w2 (consumed
        # dt-streaming by matmul2's dt-outer loop).
        nc.sync.dma_start(b1s[:], b1t[:])
        t0 = 0
        nc.sync.dma_start(xts[:, :, : blocks[0]], xt[:, :, : blocks[0]])
        for g in range(NG):
            nc.sync.dma_start(w1s[:, :, WG * g : WG * (g + 1)], w1[g])
        for blk in range(1, len(blocks)):
            t0 += blocks[blk - 1]
            nc.sync.dma_start(
                xts[:, :, t0 : t0 + blocks[blk]],
                xt[:, :, t0 : t0 + blocks[blk]],
            )
        for q in range(NQ):
            nc.sync.dma_start(w2s[:, 4 * q : 4 * (q + 1), :], w2[q])

        gpool = ctx.enter_context(tc.tile_pool(name="g", bufs=2))
        ps1 = ctx.enter_context(tc.tile_pool(name="ps1", bufs=2, space="PSUM"))
        ps2 = ctx.enter_context(tc.tile_pool(name="ps2", bufs=6, space="PSUM"))
        yev = ctx.enter_context(tc.tile_pool(name="yev", bufs=4))

        t0 = 0
        for blk, bw in enumerate(blocks):
            g = gpool.tile([128, DT, bw], bf16, tag="g")
            for dt in range(DT):
                ph = ps1.tile([128, bw], f32, tag="ph")
                for kc in range(KC):
                    nc.tensor.matmul(
                        ph[:],
                        w1s[:, kc, 128 * dt : 128 * (dt + 1)],
                        xts[:, kc, t0 : t0 + bw],
                        start=(kc == 0),
                        stop=(kc == KC - 1),
                    )
                nc.scalar.activation(
                    g[:, dt, :], ph[:],
                    mybir.ActivationFunctionType.Gelu,
                    bias=b1s[:, dt : dt + 1],
                )
            # matmul2, dt outermost: 6 concurrent PSUM accumulators, so w2
            # chunks are consumed in DMA order and g[dt] right after its GELU.
            tts = _tok_tiles(bw)
            accs = []
            for tt, tw in enumerate(tts):
                for cb in range(C // CB):
                    accs.append((
                        ps2.tile([128, CB], f32, tag="py",
                                 name=f"py_b{blk}_t{tt}_c{cb}"),
                        tt, tw, cb,
                    ))
            for dt in range(DT):
                for py, tt, tw, cb in accs:
                    nc.tensor.matmul(
                        py[:tw, :],
                        g[:, dt, TOK_TILE * tt : TOK_TILE * tt + tw],
                        w2s[:, dt, CB * cb : CB * (cb + 1)],
                        start=(dt == 0),
                        stop=(dt == DT - 1),
                    )
            # one bf16 output tile per token tile: both C halves evicted on
            # DVE, then a single DMA per 128 tokens
            ytiles = {}
            for py, tt, tw, cb in accs:
                if tt not in ytiles:
                    ytiles[tt] = yev.tile([128, C], bf16, tag="yt",
                                          name=f"yt_b{blk}_t{tt}")
                nc.vector.tensor_copy(
                    ytiles[tt][:tw, CB * cb : CB * (cb + 1)], py[:tw, :]
                )
            for tt, tw in enumerate(tts):
                r0 = t0 + TOK_TILE * tt
                nc.sync.dma_start(y[r0 : r0 + tw, :], ytiles[tt][:tw, :])
            t0 += bw
    return nc


def _route(xf, w_router):
    """Host router: softmax over experts, top-2 (jax tie semantics:
    stable, lower index first), renormalize."""
    logits = xf @ w_router.T                       # [N, E] fp32
    m = logits.max(axis=-1, keepdims=True)
    p = np.exp(logits - m)
    p /= p.sum(axis=-1, keepdims=True)
    topi = np.argsort(-p, axis=-1, kind="stable")[:, :TOP_K]   # [N, 2]
    topw = np.take_along_axis(p, topi, axis=-1)
    topw = topw / topw.sum(axis=-1, keepdims=True)
    return topi.astype(np.int32), topw.astype(np.float32)


def _run_spmd(nc, in_maps, trace=False, trace_cores=None, tmpdir=None):
    from concourse.bass_utils import run_bass_kernel_spmd

    return run_bass_kernel_spmd(
        nc, in_maps, core_ids=list(range(E)),
        trace=trace, trace_cores=trace_cores, tmpdir=tmpdir,
    )


# test.py hooks: set TRACE=True (and optionally TRACE_CORES/TRACE_DIR)
# before calling kernel() to capture an NTFF profile of the run.
TRACE = False
TRACE_CORES = None
TRACE_DIR = None
LAST_RESULT = None


def kernel(x, w_router, w1, b1, w2, b2):
    global LAST_RESULT
    x = np.asarray(x, dtype=np.float32)
    w_router = np.asarray(w_router, dtype=np.float32)
    w1 = np.asarray(w1, dtype=np.float32)
    b1 = np.asarray(b1, dtype=np.float32)
    w2 = np.asarray(w2, dtype=np.float32)
    b2 = np.asarray(b2, dtype=np.float32)

    xf = x.reshape(N, C)
    topi, topw = _route(xf, w_router)

    # token rows routed to each expert (each token appears in exactly 2)
    sel = [np.nonzero((topi == e).any(axis=-1))[0] for e in range(E)]
    max_cnt = max(len(s) for s in sel)
    cap = max(128, -(-max_cnt // 64) * 64)

    if cap not in _cache:
        _cache[cap] = _build(cap)
    nc = _cache[cap]

    bf16 = ml_dtypes.bfloat16
    xf_bf = xf.astype(bf16)
    in_maps = []
    for e in range(E):
        rows = sel[e]
        xt = np.zeros((C, cap), dtype=bf16)
        xt[:, : len(rows)] = xf_bf[rows].T
        # layouts documented in _build
        xtt = np.ascontiguousarray(
            xt.reshape(C // 128, 128, cap).transpose(1, 0, 2))
        w1t = w1[e].astype(bf16).reshape(C // 128, 128, D // 512, 512)
        w1t = np.ascontiguousarray(w1t.transpose(2, 1, 0, 3))
        w2t = w2[e].astype(bf16).reshape(D // 512, 4, 128, C)
        w2t = np.ascontiguousarray(w2t.transpose(0, 2, 1, 3))
        in_maps.append({
            "xt": xtt,
            "w1": w1t,
            "w2": w2t,
            "b1t": np.ascontiguousarray(b1[e].reshape(D // 128, 128).T),
        })

    res = _run_spmd(nc, in_maps, trace=TRACE, trace_cores=TRACE_CORES,
                    tmpdir=TRACE_DIR)
    LAST_RESULT = res

    out = np.zeros((N, C), dtype=np.float32)
    for e in range(E):
        rows = sel[e]
        if len(rows) == 0:
            continue
        ye = np.asarray(res.results[e]["y"], dtype=np.float32)[: len(rows)]
        # weight of expert e for each selected token
        is_e = topi[rows] == e               # [n_e, 2]
        wgt = (topw[rows] * is_e).sum(axis=-1)
        out[rows] += wgt[:, None] * ye
    # b2 enters after the expert matmul, inside the weighted combine
    out += (topw[:, :, None] * b2[topi]).sum(axis=1)
    return out.reshape(B, T, C)


# revision 14
# speedup vs baseline: 1.1291x; 1.0629x over previous
"""MoE layer (top-2 of 8 experts, exact GELU) on 8 Trainium2 NeuronCores.

Strategy: expert parallelism. The router (0.006% of the FLOPs) runs on the
host; each core holds one expert's weights resident in SBUF and runs the
two big matmuls for the tokens routed to that expert:

    H^T = W1[e]^T @ X_e^T          (PE, bf16, accumulated over C in fp32)
    G   = GELU(H^T + b1)           (ACT, fused bias, bf16 out)
    Y   = G^T @ W2[e]              (PE, bf16, accumulated over D in fp32)

The host dispatches each expert's token batch pre-transposed ([C, cap]) in
bf16, then combines Y across the two selected experts per token with the
renormalized top-2 weights (plus the b2 term) in fp32.

Computing H transposed (d on partitions) makes the first matmul's output
directly usable as the second matmul's stationary operand -- no on-device
transposes anywhere.
"""

import numpy as np
import ml_dtypes

B, T, C, D, E = 2, 2048, 1024, 4096, 8
N = B * T
TOP_K = 2
NT_BLOCK = 384          # token block width (matmul1 free dim)
TOK_TILE = 128          # token tile (matmul2 stationary free dim / psum partitions)
CB = 512                # output-channel tile (matmul2 free dim)

_cache = {}


MAX_WAITS = 1  # this walrus build rejects >1 sync wait per instruction


def _install_tile_patch():
    """This container's walrus build rejects >MAX_WAITS sync waits on one
    instruction ("Too many sync wait commands"). Two fixes:
    1. The stock TileContext tail puts every outstanding proc-clock wait on
       a single Drain -- split across sync-engine NOPs, one wait each.
    2. Body instructions can come out of sem assignment with 3+ waits --
       peel the excess onto same-engine NOPs inserted just before."""
    import bass_rust
    import concourse.mybir as mybir
    from concourse import tile
    from concourse.vector_clock import ScopedClock

    if getattr(tile.TileContext, "_drain_patched", False):
        return

    def _patched(self, tick_clock, wait_clock):
        nc = self.nc
        ticks = list(tick_clock.global_clock)
        for p, t in enumerate(ticks):
            if t <= 0:
                continue
            vc = bass_rust.VectorClock()
            vc.require_at_least(p, t)
            nop = nc.sync.nop(nofuse=True, hint=f"tail_wait_p{p}")
            wait_clock.add_sem_waits(nop.ins, ScopedClock({None: vc}))
        nc.sync.drain()
        nc.all_engine_barrier()
        assert self.sems is not None
        popped = nc._tile_sem_poison_stack.pop()
        assert popped is self._sem_poison
        nc.clear_and_free_semaphores(list(self.sems.allocated().values()))
        nc.all_engine_barrier()

    tile.TileContext._drain_and_barrier = _patched

    orig_lower = tile.TileContext._lower_ordered_insts

    def _split_excess_waits(self, ordered):
        nc = self.nc
        for bb_name, insts in ordered.items():
            out = []
            for inst in insts:
                si = getattr(inst, "sync_info", None)
                if (
                    si is not None
                    and si.on_wait
                    and len(si.on_wait) > MAX_WAITS
                    and isinstance(inst, mybir.Instruction)
                    and inst.engine != mybir.EngineType.Unassigned
                ):
                    waits = list(si.on_wait)
                    excess, keep = waits[:-MAX_WAITS], waits[-MAX_WAITS:]
                    while excess:
                        chunk, excess = excess[:MAX_WAITS], excess[MAX_WAITS:]
                        nop = mybir.InstNoOp(
                            name=nc.get_next_instruction_name(),
                            sync_info=mybir.SyncInfo(on_wait=chunk, on_update=[]),
                            bass_nofuse=True,
                            engine=inst.engine,
                        )
                        nc.register_instruction(nop)
                        out.append(nop)
                    inst.sync_info = mybir.SyncInfo(
                        on_wait=keep, on_update=list(si.on_update or [])
                    )
                out.append(inst)
            insts[:] = out
        return orig_lower(self, ordered)

    tile.TileContext._lower_ordered_insts = _split_excess_waits
    tile.TileContext._drain_patched = True


def _blocks_of(cap):
    """Token-block widths covering cap (multiple of 64). Full NT_BLOCK
    blocks plus one remainder block."""
    blocks = [NT_BLOCK] * (cap // NT_BLOCK)
    if cap % NT_BLOCK:
        blocks.append(cap % NT_BLOCK)
    return blocks


def _tok_tiles(width):
    """Split a block width into matmul2 stationary tiles (<=128 each)."""
    tiles = [TOK_TILE] * (width // TOK_TILE)
    if width % TOK_TILE:
        tiles.append(width % TOK_TILE)
    return tiles


def _build(cap):
    """Build the per-core Bass program for token capacity `cap`
    (a multiple of 64)."""
    import concourse.bass as bass
    import concourse.mybir as mybir
    import concourse.tile as tile
    from contextlib import ExitStack

    _install_tile_patch()

    bf16 = mybir.dt.bfloat16
    f32 = mybir.dt.float32
    KC = C // 128            # 8 contraction chunks for matmul1
    DT = D // 128            # 32 d-tiles / contraction chunks for matmul2
    WG = 512                 # w1 d-columns per DMA group (4 d-tiles)
    NG = D // WG             # 8 groups
    blocks = _blocks_of(cap)

    nc = bass.Bass()
    # Inputs are host-pre-tiled so every DMA is one big partition-major
    # transfer (~1 MB): small DMAs serialize at ~600 ns each on the sync
    # queue and starve the PE.
    #   xt : [128, KC, cap]   xt[p, kc, t]  = x^T[kc*128+p, t]
    #   w1 : [NG, 128, KC, WG] w1[g, p, kc, j] = w1[kc*128+p, g*WG+j]
    #   w2 : [NQ, 128, 4, C]  w2[q, p, a, c] = w2[(4q+a)*128+p, c]
    NQ = DT // 4
    xt = nc.declare_dram_parameter("xt", [128, KC, cap], bf16, isOutput=False)
    w1 = nc.declare_dram_parameter("w1", [NG, 128, KC, WG], bf16, isOutput=False)
    w2 = nc.declare_dram_parameter("w2", [NQ, 128, 4, C], bf16, isOutput=False)
    b1t = nc.declare_dram_parameter("b1t", [128, DT], f32, isOutput=False)
    # output is y^T, cb-major: y[cb, p, t] = y^T[cb*128+p, t]
    y = nc.declare_dram_parameter("y", [C // 128, 128, cap], bf16, isOutput=True)

    from concourse.bass import _add_dep_helper

    with tile.TileContext(nc) as tc, ExitStack() as ctx:
        const = ctx.enter_context(tc.tile_pool(name="const", bufs=1))
        w1s = const.tile([128, KC, D], bf16)
        w2s = const.tile([128, DT, C], bf16)
        xts = const.tile([128, KC, cap], bf16)
        b1s = const.tile([128, DT], f32)

        # DMA emission order == consumption order (priority for the
        # scheduler): bias, block-0 activations, w1 by d-group (matmul1
        # consumes d-major), remaining activations, then w2 (consumed
        # dt-streaming by matmul2's dt-outer loop). The explicit dep edges
        # keep late-needed transfers off the HBM bus while the PE is
        # starved waiting for block-0 activations + the first w1 group.
        nc.sync.dma_start(b1s[:], b1t[:])
        t0 = 0
        nc.sync.dma_start(xts[:, :, : blocks[0]], xt[:, :, : blocks[0]])
        w1_dmas = []
        for g in range(NG):
            w1_dmas.append(
                nc.sync.dma_start(w1s[:, :, WG * g : WG * (g + 1)], w1[g]))
        for blk in range(1, len(blocks)):
            t0 += blocks[blk - 1]
            dma = nc.sync.dma_start(
                xts[:, :, t0 : t0 + blocks[blk]],
                xt[:, :, t0 : t0 + blocks[blk]],
            )
            _add_dep_helper(dma.ins, w1_dmas[1].ins, sync=True,
                            reason="xt tail after early w1 groups")
        for q in range(NQ):
            dma = nc.sync.dma_start(w2s[:, 4 * q : 4 * (q + 1), :], w2[q])
            _add_dep_helper(dma.ins, w1_dmas[3].ins, sync=True,
                            reason="w2 after w1 half in")

        NB = C // 128          # 8 output-channel tiles for matmul2
        gpool = ctx.enter_context(tc.tile_pool(name="g", bufs=2))
        ps1 = ctx.enter_context(tc.tile_pool(name="ps1", bufs=2, space="PSUM"))
        ps2 = ctx.enter_context(tc.tile_pool(name="ps2", bufs=4, space="PSUM"))
        yev = ctx.enter_context(tc.tile_pool(name="yev", bufs=2))

        t0 = 0
        for blk, bw in enumerate(blocks):
            g = gpool.tile([128, DT, bw], bf16, tag="g")
            for dt in range(DT):
                ph = ps1.tile([128, bw], f32, tag="ph")
                for kc in range(KC):
                    nc.tensor.matmul(
                        ph[:],
                        w1s[:, kc, 128 * dt : 128 * (dt + 1)],
                        xts[:, kc, t0 : t0 + bw],
                        start=(kc == 0),
                        stop=(kc == KC - 1),
                    )
                nc.scalar.activation(
                    g[:, dt, :], ph[:],
                    mybir.ActivationFunctionType.Gelu,
                    bias=b1s[:, dt : dt + 1],
                )
            # matmul2 computes y^T: lhsT = w2 tile (stationary), rhs = g
            # (tokens moving). Block 0 runs dt-outer in two 4-cb passes so
            # w2 chunks are consumed in DMA order; later blocks run
            # cb-outer so evictions overlap the remaining matmuls.
            yt = yev.tile([128, NB, bw], bf16, tag="yt")
            if blk == 0:
                for half in range(2):
                    cbs = range(4 * half, 4 * half + 4)
                    pys = {cb: ps2.tile([128, bw], f32, tag="py",
                                        name=f"py_b{blk}_c{cb}")
                           for cb in cbs}
                    for dt in range(DT):
                        for cb in cbs:
                            nc.tensor.matmul(
                                pys[cb][:],
                                w2s[:, dt, 128 * cb : 128 * (cb + 1)],
                                g[:, dt, :],
                                start=(dt == 0),
                                stop=(dt == DT - 1),
                            )
                    for cb in cbs:
                        nc.vector.tensor_copy(yt[:, cb, :], pys[cb][:])
            else:
                for cb in range(NB):
                    py = ps2.tile([128, bw], f32, tag="py",
                                  name=f"py_b{blk}_c{cb}")
                    for dt in range(DT):
                        nc.tensor.matmul(
                            py[:],
                            w2s[:, dt, 128 * cb : 128 * (cb + 1)],
                            g[:, dt, :],
                            start=(dt == 0),
                            stop=(dt == DT - 1),
                        )
                    nc.vector.tensor_copy(yt[:, cb, :], py[:])
            if blk == len(blocks) - 1:
                # split the final writeback so half overlaps the last matmuls
                for h in range(2):
                    nc.sync.dma_start(
                        y[4 * h : 4 * h + 4, :, t0 : t0 + bw].rearrange(
                            "cb p t -> p cb t"),
                        yt[:, 4 * h : 4 * h + 4, :],
                    )
            else:
                nc.sync.dma_start(
                    y[:, :, t0 : t0 + bw].rearrange("cb p t -> p cb t"), yt[:]
                )
            t0 += bw
    return nc


def _route(xf, w_router):
    """Host router: softmax over experts, top-2 (jax tie semantics:
    stable, lower index first), renormalize."""
    logits = xf @ w_router.T                       # [N, E] fp32
    m = logits.max(axis=-1, keepdims=True)
    p = np.exp(logits - m)
    p /= p.sum(axis=-1, keepdims=True)
    topi = np.argsort(-p, axis=-1, kind="stable")[:, :TOP_K]   # [N, 2]
    topw = np.take_along_axis(p, topi, axis=-1)
    topw = topw / topw.sum(axis=-1, keepdims=True)
    return topi.astype(np.int32), topw.astype(np.float32)


def _run_spmd(nc, in_maps, trace=False, trace_cores=None, tmpdir=None):
    from concourse.bass_utils import run_bass_kernel_spmd

    return run_bass_kernel_spmd(
        nc, in_maps, core_ids=list(range(E)),
        trace=trace, trace_cores=trace_cores, tmpdir=tmpdir,
    )


# test.py hooks: set TRACE=True (and optionally TRACE_CORES/TRACE_DIR)
# before calling kernel() to capture an NTFF profile of the run.
TRACE = False
TRACE_CORES = None
TRACE_DIR = None
LAST_RESULT = None


def kernel(x, w_router, w1, b1, w2, b2):
    global LAST_RESULT
    x = np.asarray(x, dtype=np.float32)
    w_router = np.asarray(w_router, dtype=np.float32)
    w1 = np.asarray(w1, dtype=np.float32)
    b1 = np.asarray(b1, dtype=np.float32)
    w2 = np.asarray(w2, dtype=np.float32)
    b2 = np.asarray(b2, dtype=np.float32)

    xf = x.reshape(N, C)
    topi, topw = _route(xf, w_router)

    # token rows routed to each expert (each token appears in exactly 2)
    sel = [np.nonzero((topi == e).any(axis=-1))[0] for e in range(E)]
    max_cnt = max(len(s) for s in sel)
    cap = max(128, -(-max_cnt // 64) * 64)

    if cap not in _cache:
        _cache[cap] = _build(cap)
    nc = _cache[cap]

    bf16 = ml_dtypes.bfloat16
    xf_bf = xf.astype(bf16)
    in_maps = []
    for e in range(E):
        rows = sel[e]
        xt = np.zeros((C, cap), dtype=bf16)
        xt[:, : len(rows)] = xf_bf[rows].T
        # layouts documented in _build
        xtt = np.ascontiguousarray(
            xt.reshape(C // 128, 128, cap).transpose(1, 0, 2))
        w1t = w1[e].astype(bf16).reshape(C // 128, 128, D // 512, 512)
        w1t = np.ascontiguousarray(w1t.transpose(2, 1, 0, 3))
        w2t = w2[e].astype(bf16).reshape(D // 512, 4, 128, C)
        w2t = np.ascontiguousarray(w2t.transpose(0, 2, 1, 3))
        in_maps.append({
            "xt": xtt,
            "w1": w1t,
            "w2": w2t,
            "b1t": np.ascontiguousarray(b1[e].reshape(D // 128, 128).T),
        })

    res = _run_spmd(nc, in_maps, trace=TRACE, trace_cores=TRACE_CORES,
                    tmpdir=TRACE_DIR)
    LAST_RESULT = res

    out = np.zeros((N, C), dtype=np.float32)
    for e in range(E):
        rows = sel[e]
        if len(rows) == 0:
            continue
        ye = np.asarray(res.results[e]["y"], dtype=np.float32)
        ye = ye.reshape(C, -1).T[: len(rows)]          # y^T, cb-major -> [n, C]
        # weight of expert e for each selected token
        is_e = topi[rows] == e               # [n_e, 2]
        wgt = (topw[rows] * is_e).sum(axis=-1)
        out[rows] += wgt[:, None] * ye
    # b2 enters after the expert matmul, inside the weighted combine
    out += (topw[:, :, None] * b2[topi]).sum(axis=1)
    return out.reshape(B, T, C)
